# revision 5
# baseline (speedup 1.0000x reference)
"""Multi-head attention (B=4, S=2048, D=1024, H=16, Dk=64) on 8 TRN2 cores.

Sharding: core c handles batch b = c // 2 and head-group hg = c % 2
(8 heads = 512 output columns each).  Each core projects its batch's
Q/K/V with its head-group's weight columns, computes scores/softmax/
context for its 8 heads, and writes attn[b, hg*8:(hg+1)*8] plus
context[b, :, hg*512:(hg+1)*512].

Per-core plan (single-exp design, all engines overlapped):
  - x^T built via PE transpose (fp32 has no DMA transpose); Q^T/K^T
    projected into [e, s] layout (float32r, full-rate PE); V into
    [s, per-head 64+ones] layout for the context matmul.
  - scores computed ONCE per head-block in [k, q] layout; exp on ACT
    (batched N=1024 to amortize the 352-cycle ACT issue overhead).
  - context^T (and softmax row-sums, via the ones column) accumulated
    over k in PSUM in one matmul chain; PE-transposed back to [q, .].
  - attn output = PE-transpose of exp tiles, normalized on DVE
    (PSUM -> SBUF) and DMA'd out in 2 MB chunks.
"""

import numpy as np

import concourse.mybir as mybir
import concourse.tile as tile
from concourse import bacc
from concourse.bass_utils import run_bass_kernel_spmd
from concourse.masks import make_identity

F32 = mybir.dt.float32
F32R = mybir.dt.float32r
BF16 = mybir.dt.bfloat16
AF = mybir.ActivationFunctionType
AX = mybir.AxisListType

B, S, D = 4, 2048, 1024
HEAD, DKV = 16, 64
NCORES = 8
HL = HEAD // 2          # heads per core (head-group of 8)
E = HL * DKV            # 512: per-core QKV width
P = 128

MM_DT = F32R            # dtype for Q/K-side matmuls (rounded fp32, full rate)
EXP_DT = BF16           # dtype for exp'd scores / V (context + attn path)


def emit_attention(tc, outs, ins):
    """Emit the per-core kernel. outs/ins are dicts of DRAM APs."""
    from contextlib import ExitStack

    nc = tc.nc
    xq, xk, xv = ins["xq"], ins["xk"], ins["xv"]
    wq, wk, wv = ins["wq"], ins["wk"], ins["wv"]
    attn_o, ctx_o = outs["attn_o"], outs["ctx_o"]

    with ExitStack() as outer:
        singles = outer.enter_context(tc.tile_pool(name="singles", bufs=1))
        qkv_pool = outer.enter_context(tc.tile_pool(name="qkv", bufs=1))

        ident = singles.tile([P, P], F32)
        make_identity(nc, ident)
        identb = singles.tile([P, P], EXP_DT)
        make_identity(nc, identb)

        # Q^T/K^T: [p, et, s] holds row e = et*128 + p (rounded to MM_DT).
        # V: [p, st, h, 65] holds row s = st*128 + p; col 64 is the ones
        # column that makes the context matmul also produce softmax sums.
        qT = qkv_pool.tile([P, 4, S], MM_DT, tag="qT")
        kT = qkv_pool.tile([P, 4, S], MM_DT, tag="kT")
        vS = qkv_pool.tile([P, 16, HL, DKV + 1], EXP_DT, tag="vS")
        nc.vector.memset(vS[:, :, :, DKV:DKV + 1], 1.0)

        # ---------------- Phase B: QKV projections ----------------
        with ExitStack() as ph:
            wpool = ph.enter_context(tc.tile_pool(name="w", bufs=1))
            wraw = ph.enter_context(tc.tile_pool(name="wraw", bufs=2))
            xtp = ph.enter_context(tc.tile_pool(name="xT", bufs=1))
            ldp = ph.enter_context(tc.tile_pool(name="ld", bufs=3))
            psT = ph.enter_context(tc.tile_pool(name="psT", bufs=4, space="PSUM"))
            psP = ph.enter_context(tc.tile_pool(name="psP", bufs=3, space="PSUM"))

            for ti, (x_ap, w_ap) in enumerate(((xq, wq), (xk, wk), (xv, wv))):
                w_sb = wpool.tile([P, 8, E], MM_DT, tag="w")
                for wc in range(4):
                    wr = wraw.tile([P, 2, E], F32, tag="wraw")
                    nc.sync.dma_start(
                        out=wr,
                        in_=w_ap.rearrange("(dc p) e -> p dc e", p=P)[
                            :, wc * 2:(wc + 1) * 2, :
                        ],
                    )
                    nc.vector.tensor_copy(out=w_sb[:, wc * 2:(wc + 1) * 2, :], in_=wr)
                xT = xtp.tile([P, 8, S], MM_DT, tag="xT")
                for st in range(16):
                    ld = ldp.tile([P, D], F32, tag="ld")
                    nc.sync.dma_start(out=ld, in_=x_ap[st * P:(st + 1) * P, :])
                    for dc in range(8):
                        pt = psT.tile([P, P], F32, tag="psT")
                        nc.tensor.transpose(pt, ld[:, dc * P:(dc + 1) * P], ident)
                        nc.vector.tensor_copy(
                            out=xT[:, dc, st * P:(st + 1) * P], in_=pt
                        )
                if ti < 2:  # Q or K -> [e, s] layout
                    dst = qT if ti == 0 else kT
                    for et in range(4):
                        for sc in range(4):
                            pp = psP.tile([P, 512], F32, tag="psP")
                            for dc in range(8):
                                nc.tensor.matmul(
                                    pp,
                                    lhsT=w_sb[:, dc, et * P:(et + 1) * P],
                                    rhs=xT[:, dc, sc * 512:(sc + 1) * 512],
                                    start=(dc == 0),
                                    stop=(dc == 7),
                                )
                            nc.vector.tensor_copy(
                                out=dst[:, et, sc * 512:(sc + 1) * 512], in_=pp
                            )
                else:  # V -> [s, h, dk] layout with ones column
                    for st in range(16):
                        pp = psP.tile([P, 512], F32, tag="psP")
                        for dc in range(8):
                            nc.tensor.matmul(
                                pp,
                                lhsT=xT[:, dc, st * P:(st + 1) * P],
                                rhs=w_sb[:, dc, :],
                                start=(dc == 0),
                                stop=(dc == 7),
                            )
                        for h in range(HL):
                            nc.vector.tensor_copy(
                                out=vS[:, st, h, 0:DKV],
                                in_=pp[:, h * DKV:(h + 1) * DKV],
                            )

        # ---------------- Phase C: attention ----------------
        with ExitStack() as ph:
            ctxp = ph.enter_context(tc.tile_pool(name="ctx", bufs=2))
            attnp = ph.enter_context(tc.tile_pool(name="attn", bufs=2))
            expTp = ph.enter_context(tc.tile_pool(name="expT", bufs=2))
            smll = ph.enter_context(tc.tile_pool(name="small", bufs=3))
            psA = ph.enter_context(tc.tile_pool(name="psA", bufs=2, space="PSUM"))
            psC = ph.enter_context(tc.tile_pool(name="psC", bufs=1, space="PSUM"))
            psD = ph.enter_context(tc.tile_pool(name="psD", bufs=2, space="PSUM"))
            psG = ph.enter_context(tc.tile_pool(name="psG", bufs=1, space="PSUM"))

            for qbg in range(4):  # global q block of 512 rows
                ctx_t = ctxp.tile([P, 4, E], F32, tag="ctx")
                for h in range(HL):
                    et, bp = h // 2, 64 * (h % 2)
                    kT_h = kT[bp:bp + 64, et, :]
                    qT_h = qT[bp:bp + 64, et, :]
                    qcols = qT_h[:, qbg * 512:(qbg + 1) * 512]

                    # scores^T [k, q] -> exp (EXP_DT), batched 2 k-tiles/ACT op
                    expT = expTp.tile([P, 16, 512], EXP_DT, tag="expT")
                    for ktg in range(8):
                        pa = psA.tile([P, 2, 512], F32, tag="psA")
                        for j in (0, 1):
                            kt = ktg * 2 + j
                            nc.tensor.matmul(
                                pa[:, j, :],
                                lhsT=kT_h[:, kt * P:(kt + 1) * P],
                                rhs=qcols,
                                start=True,
                                stop=True,
                            )
                        nc.scalar.activation(
                            out=expT[:, ktg * 2:(ktg + 1) * 2, :],
                            in_=pa,
                            func=AF.Exp,
                            scale=0.125,
                        )

                    # context^T [dk, q] + row sums (ones column), over k
                    pc = psC.tile([DKV + 1, 512], F32, tag="psC")
                    for kt in range(16):
                        nc.tensor.matmul(
                            pc,
                            lhsT=vS[:, kt, h, :],
                            rhs=expT[:, kt, :],
                            start=(kt == 0),
                            stop=(kt == 15),
                        )
                    ctxT = smll.tile([DKV + 1, 512], F32, tag="ctxT")
                    nc.vector.tensor_copy(out=ctxT, in_=pc)
                    # transpose [65, q] -> [q, 65] per 128-q subtile
                    pg = psG.tile([P, 4, DKV + 1], F32, tag="psG")
                    for qs in range(4):
                        nc.tensor.transpose(
                            pg[:, qs, :],
                            ctxT[:, qs * P:(qs + 1) * P],
                            ident[0:DKV + 1, 0:DKV + 1],
                        )
                    invs = smll.tile([P, 4], F32, tag="invs")
                    nc.vector.reciprocal(out=invs, in_=pg[:, :, DKV])
                    for qs in range(4):
                        nc.vector.tensor_scalar_mul(
                            out=ctx_t[:, qs, h * DKV:(h + 1) * DKV],
                            in0=pg[:, qs, 0:DKV],
                            scalar1=invs[:, qs:qs + 1],
                        )

                    # attn [q, k] = transpose(expT) * inv, in 256-row halves
                    for half in range(2):
                        attn_sb = attnp.tile([P, 2, S], F32, tag="attn")
                        for qs2 in range(2):
                            qs = half * 2 + qs2
                            for ktg in range(4):
                                pd = psD.tile([P, 512], EXP_DT, tag="psD")
                                for k4 in range(4):
                                    kt = ktg * 4 + k4
                                    nc.tensor.transpose(
                                        pd[:, k4 * P:(k4 + 1) * P],
                                        expT[:, kt, qs * P:(qs + 1) * P],
                                        identb,
                                    )
                                nc.vector.tensor_scalar_mul(
                                    out=attn_sb[:, qs2, ktg * 512:(ktg + 1) * 512],
                                    in0=pd,
                                    scalar1=invs[:, qs:qs + 1],
                                )
                        nc.sync.dma_start(
                            out=attn_o[
                                h,
                                qbg * 512 + half * 256: qbg * 512 + (half + 1) * 256,
                                :,
                            ].rearrange("(t p) k -> p t k", p=P),
                            in_=attn_sb,
                        )
                nc.sync.dma_start(
                    out=ctx_o[qbg * 512:(qbg + 1) * 512, :].rearrange(
                        "(t p) e -> p t e", p=P
                    ),
                    in_=ctx_t,
                )


def build_nc():
    nc = bacc.Bacc("TRN2", target_bir_lowering=False, debug=False)
    ins = {
        "xq": nc.dram_tensor("xq", [S, D], F32, kind="ExternalInput").ap(),
        "xk": nc.dram_tensor("xk", [S, D], F32, kind="ExternalInput").ap(),
        "xv": nc.dram_tensor("xv", [S, D], F32, kind="ExternalInput").ap(),
        "wq": nc.dram_tensor("wq", [D, E], F32, kind="ExternalInput").ap(),
        "wk": nc.dram_tensor("wk", [D, E], F32, kind="ExternalInput").ap(),
        "wv": nc.dram_tensor("wv", [D, E], F32, kind="ExternalInput").ap(),
    }
    outs = {
        "attn_o": nc.dram_tensor("attn_o", [HL, S, S], F32, kind="ExternalOutput").ap(),
        "ctx_o": nc.dram_tensor("ctx_o", [S, E], F32, kind="ExternalOutput").ap(),
    }
    with tile.TileContext(nc) as tc:
        emit_attention(tc, outs, ins)
    nc.compile()
    return nc


def make_in_maps(input_Q, input_K, input_V, W_Q, W_K, W_V):
    in_maps = []
    for c in range(NCORES):
        b, hg = c // 2, c % 2
        cols = slice(hg * E, (hg + 1) * E)
        in_maps.append(
            {
                "xq": np.ascontiguousarray(input_Q[b], dtype=np.float32),
                "xk": np.ascontiguousarray(input_K[b], dtype=np.float32),
                "xv": np.ascontiguousarray(input_V[b], dtype=np.float32),
                "wq": np.ascontiguousarray(W_Q[:, cols], dtype=np.float32),
                "wk": np.ascontiguousarray(W_K[:, cols], dtype=np.float32),
                "wv": np.ascontiguousarray(W_V[:, cols], dtype=np.float32),
            }
        )
    return in_maps


def assemble(results):
    attn = np.empty((B, HEAD, S, S), np.float32)
    ctx = np.empty((B, S, HEAD * DKV), np.float32)
    for c, r in enumerate(results):
        b, hg = c // 2, c % 2
        attn[b, hg * HL:(hg + 1) * HL] = r["attn_o"]
        ctx[b, :, hg * E:(hg + 1) * E] = r["ctx_o"]
    return ctx, attn


def kernel(input_Q, input_K, input_V, W_Q, W_K, W_V):
    input_Q = np.asarray(input_Q)
    input_K = np.asarray(input_K)
    input_V = np.asarray(input_V)
    W_Q = np.asarray(W_Q)
    W_K = np.asarray(W_K)
    W_V = np.asarray(W_V)
    nc = build_nc()
    in_maps = make_in_maps(input_Q, input_K, input_V, W_Q, W_K, W_V)
    res = run_bass_kernel_spmd(nc, in_maps, core_ids=list(range(NCORES)))
    return assemble(res.results)
